# revision 1
# baseline (speedup 1.0000x reference)
"""GP marginal log-likelihood kernel for Trainium2 (Bass/Tile).

Computes -0.5 * y^T A^-1 y - 0.5 * logdet(A) for A = K + sigma^2 I where
K is the RBF covariance on the integer grid 0..T-1 (T=8192).

A is symmetric positive-definite *Toeplitz* and effectively *banded*
(entries vanish below f32 eps for |i-j| > 255 at lengthscale 32), and is
well conditioned: eig(A) in [sigma^2, sigma^2 + v*sum_d k(d)] (~[1, 81.2]).
This kernel exploits that structure instead of doing a dense 8192^3
factorization:

  * quad = y^T A^-1 y: x = p(A) y where p is a least-squares-optimal
    polynomial fitted (on the host, from the hyperparameters alone) to the
    *known* model spectrum of A -- the symbol samples f(2 pi j / T) -- and
    applied on device in the numerically stable Chebyshev basis:
        v_{m+1} = (2 As) v_m - v_{m-1},   x = sum_m gamma_m v_m,
    where each (2 As) v is a block-pentadiagonal matvec: 5 tensor-engine
    matmuls with 128x128 stationary band blocks.  The second-order
    functional quad = x^T (2y - A x) makes the final error quadratic in
    the solver error (~1e-5 relative at degree 18).
  * logdet via the strong Szego limit theorem:
        logdet A = T*c_0 + sum_{k>=1} k*c_k^2,   c_k = Fourier coeffs of
    log f(theta), f = the symbol of A.  For an analytic positive symbol the
    remainder decays like exp(-2*beta*T); at T=8192 it is far below f32 eps
    (verified numerically: < 1e-11 in f64, < 5e-6 in f32).  f is evaluated
    on device in closed (Poisson-summation) form with two Exps per grid
    point; the cosine/DCT matrix is generated on device (outer-product
    matmul + exact 2^23 range reduction + Sin activation).

Everything data-dependent runs on the device.  The host only computes the
iteration coefficient schedule and a handful of scalar parameters from the
scalar hyperparameters (sigma^2, lengthscale, variance); the final scalar
is assembled on core 0 and DMA'd out.  All 8 cores run the same program on
replicated inputs (the answer is a single scalar; core 0's result is
gathered).
"""

import math

import numpy as np

T = 8192
P = 128  # partitions
NBLK = T // P  # 64 column blocks
NPAD = 2  # zero pad columns on each side of the padded vec tiles
BW = 255  # band half-width kept in the 5 block matrices
N_GRID = 512  # Szego quadrature grid size (half-grid 0..256 used)
NJ = N_GRID // 2 + 1  # 257 half-grid points
K_DCT = 256  # highest Fourier coefficient kept (c_k ~ e^{-0.031k})
KC = K_DCT + 1  # DCT output columns incl. k=0
N_JTILES = 3  # ceil(257/128): 2 full partition tiles + 1 single-row
N_DEG = 18  # polynomial degree bound for the solve (17 matvecs)
MAGIC = 8388608.0  # 2^23: x + MAGIC - MAGIC == round-to-nearest(x) in f32

_prog_cache = {}


def _ls_poly(sig2, ell, var, n_deg):
    """Host-side iteration schedule: LS-optimal solve polynomial.

    Fits p(lam) = sum_m gamma_m T_m(scaled lam) minimizing
    sum_j (1 - lam_j p(lam_j))^2 / lam_j over the model spectrum
    lam_j = f(2 pi j / T) (symbol samples, the asymptotic eigenvalue
    distribution of A).  Returns (gamma, lo, hi).  Cost: a small lstsq on
    hyperparameter-derived data only -- part of the schedule, like
    Chebyshev coefficients.
    """
    th = np.linspace(0.0, np.pi, T // 2 + 1)
    lam = sig2 + var * ell * math.sqrt(2.0 * math.pi) * (
        np.exp(-((ell * th) ** 2) / 2.0)
        + np.exp(-((ell * (th - 2 * math.pi)) ** 2) / 2.0)
    )
    lo, hi = float(lam.min()), float(lam.max())
    xs = (2.0 * lam - (hi + lo)) / (hi - lo)
    V = np.zeros((lam.size, n_deg))
    V[:, 0] = 1.0
    if n_deg > 1:
        V[:, 1] = xs
    for m in range(2, n_deg):
        V[:, m] = 2.0 * xs * V[:, m - 1] - V[:, m - 2]
    w = 1.0 / lam
    Aw = V * (lam * np.sqrt(w))[:, None]
    b = np.sqrt(w)
    g, *_ = np.linalg.lstsq(Aw, b, rcond=None)
    return g, lo, hi


def _build(sig2, ell, var, n_deg, debug=False, n_copies=1, loop_n=0):
    """Emit the full program into a fresh Bacc instance and return it."""
    import concourse.mybir as mybir
    import concourse.tile as tile
    from concourse import bacc
    from concourse.masks import make_identity

    f32 = mybir.dt.float32
    i32 = mybir.dt.int32
    AF = mybir.ActivationFunctionType
    OP = mybir.AluOpType

    gam, lam_lo, lam_hi = _ls_poly(sig2, ell, var, n_deg)
    # 2*As = sc2*A + sh2*I
    sc2 = 4.0 / (lam_hi - lam_lo)
    sh2 = -2.0 * (lam_hi + lam_lo) / (lam_hi - lam_lo)

    nc = bacc.Bacc("TRN2", target_bir_lowering=False, debug=False)
    y_dram = nc.dram_tensor("y", [T], f32, kind="ExternalInput")
    # params (rows replicated so any slice works as a per-partition scalar):
    # 0: -1/(2 l^2)   1: -v      2: -sigma^2  3: sigma^2
    # 4: v*l*sqrt(2pi)  5: -l^2/2  6,7: spare
    par_dram = nc.dram_tensor("par", [P, 8], f32, kind="ExternalInput")
    out_dram = nc.dram_tensor("out", [1, n_copies], f32, kind="ExternalOutput")
    if debug:
        dbg_c = nc.dram_tensor("dbg_c", [1, KC], f32, kind="ExternalOutput")
        dbg_x = nc.dram_tensor("dbg_x", [P, NBLK], f32, kind="ExternalOutput")
        dbg_ql = nc.dram_tensor("dbg_ql", [1, 4], f32, kind="ExternalOutput")

    with tile.TileContext(nc) as tc:
        with (
            tc.tile_pool(name="const", bufs=1) as cpool,
            tc.tile_pool(name="work", bufs=1) as wpool,
            tc.tile_pool(name="dct", bufs=2) as dpool,
            tc.tile_pool(name="ps", bufs=1, space="PSUM") as ppool,
            tc.tile_pool(name="psdct", bufs=2, space="PSUM") as pdpool,
        ):
            def emit(ci):
                _emit_one(
                    nc, tc, cpool, wpool, dpool, ppool, pdpool,
                    mybir, make_identity,
                    y_dram, par_dram, out_dram,
                    dbg_c if debug and ci == 0 else None,
                    dbg_x if debug and ci == 0 else None,
                    dbg_ql if debug and ci == 0 else None,
                    gam, sc2, sh2, n_deg, ci,
                )

            if loop_n:
                with tc.For_i(0, loop_n, 1):
                    emit(0)
            else:
                for ci in range(n_copies):
                    emit(ci)

    nc.compile()
    return nc


def _emit_one(
    nc, tc, cpool, wpool, dpool, ppool, pdpool, mybir, make_identity,
    y_dram, par_dram, out_dram, dbg_c, dbg_x, dbg_ql,
    gam, sc2, sh2, n_deg, ci,
):
    from concourse.tile_rust import add_dep_helper

    f32 = mybir.dt.float32
    i32 = mybir.dt.int32
    AF = mybir.ActivationFunctionType
    OP = mybir.AluOpType

    par = cpool.tile([P, 8], f32, tag=f"par{ci}")
    nc.sync.dma_start(par[:], par_dram[:])

    ident = cpool.tile([P, P], f32, tag=f"id{ci}")
    make_identity(nc, ident[:])
    bneg2pi = cpool.tile([P, 1], f32, tag=f"bneg2pi{ci}")
    nc.vector.memset(bneg2pi[:], -2.0 * math.pi)

    # contiguous row-major load, then PE transpose into the block layout
    # ysb[r, b] = y[b*128 + r]  (a 4B-strided DMA would be descriptor-bound)
    yrow = cpool.tile([NBLK, P], f32, tag=f"yrow{ci}")
    nc.sync.dma_start(yrow[:], y_dram.rearrange("(b r) -> b r", b=NBLK))
    ysb_ps = ppool.tile([P, NBLK], f32, tag="ysb_ps")
    nc.tensor.transpose(ysb_ps[:], yrow[:], ident[:NBLK, :NBLK])
    ysb = cpool.tile([P, NBLK], f32, tag=f"ysb{ci}")
    nc.vector.tensor_copy(ysb[:], ysb_ps[:])

    # ---------------- band block matrices ----------------
    # NS[c, m, r] = -(v*exp(-(128(m-2)+c-r)^2/(2 l^2)) + sig2*[d==0])
    # NS2 = -sc2*NS + sh2*I  (the 2*As operator blocks)
    dmat_i = cpool.tile([P, 5, P], i32, tag=f"dmi{ci}")
    nc.gpsimd.iota(
        dmat_i[:], pattern=[[P, 5], [-1, P]], base=-2 * P, channel_multiplier=1
    )
    dmat = cpool.tile([P, 5, P], f32, tag=f"dm{ci}")
    nc.gpsimd.tensor_copy(dmat[:], dmat_i[:])
    nc.scalar.activation(dmat[:], dmat[:], AF.Square)
    nc.scalar.activation(dmat[:], dmat[:], AF.Exp, scale=par[:, 0:1])
    NS = cpool.tile([P, 5, P], f32, tag=f"NS{ci}")
    nc.vector.tensor_scalar(NS[:], dmat[:], par[:, 1:2], None, op0=OP.mult)
    nc.vector.scalar_tensor_tensor(
        NS[:, 2, :],
        in0=ident[:],
        scalar=par[:, 2:3],
        in1=NS[:, 2, :],
        op0=OP.mult,
        op1=OP.add,
    )
    NS2 = cpool.tile([P, 5, P], f32, tag=f"NS2{ci}")
    nc.gpsimd.tensor_scalar(NS2[:], NS[:], float(-sc2), None, op0=OP.mult)
    nc.vector.scalar_tensor_tensor(
        NS2[:, 2, :],
        in0=ident[:],
        scalar=float(sh2),
        in1=NS2[:, 2, :],
        op0=OP.mult,
        op1=OP.add,
    )

    # ---------------- Szego logdet (phase 1) ----------------
    jmat_i = cpool.tile([P, N_JTILES], i32, tag=f"jmi{ci}")
    nc.gpsimd.iota(
        jmat_i[:], pattern=[[P, N_JTILES]], base=0, channel_multiplier=1
    )
    jmat = cpool.tile([P, N_JTILES], f32, tag=f"jm{ci}")
    nc.vector.tensor_copy(jmat[:], jmat_i[:])
    th2 = wpool.tile([P, N_JTILES], f32, tag=f"th2{ci}")
    nc.scalar.activation(th2[:], jmat[:], AF.Square, scale=2.0 * math.pi / N_GRID)
    e1 = wpool.tile([P, N_JTILES], f32, tag=f"e1{ci}")
    nc.scalar.activation(e1[:], th2[:], AF.Exp, scale=par[:, 5:6])
    th2b = wpool.tile([P, N_JTILES], f32, tag=f"th2b{ci}")
    nc.scalar.activation(
        th2b[:],
        jmat[:],
        AF.Square,
        scale=2.0 * math.pi / N_GRID,
        bias=bneg2pi[:],
    )
    e2 = wpool.tile([P, N_JTILES], f32, tag=f"e2{ci}")
    nc.scalar.activation(e2[:], th2b[:], AF.Exp, scale=par[:, 5:6])
    fsym = wpool.tile([P, N_JTILES], f32, tag=f"fsym{ci}")
    nc.vector.tensor_tensor(fsym[:], e1[:], e2[:], op=OP.add)
    nc.vector.tensor_scalar(
        fsym[:], fsym[:], par[:, 4:5], par[:, 3:4], op0=OP.mult, op1=OP.add
    )
    gl = wpool.tile([P, N_JTILES], f32, tag=f"gl{ci}")
    nc.scalar.activation(gl[:], fsym[:], AF.Ln)
    wq = cpool.tile([P, N_JTILES], f32, tag=f"wq{ci}")
    nc.gpsimd.memset(wq[:], 2.0 / N_GRID)
    nc.gpsimd.memset(wq[:, N_JTILES - 1 : N_JTILES], 0.0)
    nc.gpsimd.memset(wq[0:1, 0:1], 1.0 / N_GRID)
    nc.gpsimd.memset(wq[0:1, N_JTILES - 1 : N_JTILES], 1.0 / N_GRID)
    nc.vector.tensor_tensor(gl[:], gl[:], wq[:], op=OP.mult)

    # DCT: c[k] = sum_j g~[j] cos(2 pi j k / N)
    kvec_i = cpool.tile([1, KC], i32, tag=f"kvi{ci}")
    nc.gpsimd.iota(kvec_i[:], pattern=[[1, KC]], base=0, channel_multiplier=0)
    kvec = cpool.tile([1, KC], f32, tag=f"kv{ci}")
    nc.vector.tensor_copy(kvec[:], kvec_i[:])
    kdivn = cpool.tile([1, KC], f32, tag=f"kdn{ci}")
    nc.vector.tensor_scalar(kdivn[:], kvec[:], 1.0 / N_GRID, None, op0=OP.mult)

    c_ps = ppool.tile([1, KC], f32, tag="c_ps")
    for t in range(N_JTILES):
        rows = P if t < N_JTILES - 1 else 1
        jv_i = dpool.tile([1, P], i32, tag="jv_i")
        nc.gpsimd.iota(
            jv_i[:1, :rows], pattern=[[1, rows]], base=t * P, channel_multiplier=0
        )
        jv = dpool.tile([1, P], f32, tag="jv")
        nc.vector.tensor_copy(jv[:1, :rows], jv_i[:1, :rows])
        tau_ps = pdpool.tile([P, KC], f32, tag="tau_ps")
        nc.tensor.matmul(
            tau_ps[:rows, :],
            jv[:1, :rows],
            kdivn[:],
            start=True,
            stop=True,
            skip_group_check=True,
        )
        # a1 = tau + 0.25; R = round(a1) via +-2^23 (ACT, rne adds);
        # psi = a1 - R in [-0.5, 0.5];  cos(2 pi tau) = Sin(2 pi psi)
        a1 = dpool.tile([P, KC], f32, tag="a1")
        nc.vector.tensor_scalar(
            a1[:rows, :], tau_ps[:rows, :], 0.25, None, op0=OP.add
        )
        rnd0 = dpool.tile([P, KC], f32, tag="rnd0")
        nc.scalar.activation(rnd0[:rows, :], a1[:rows, :], AF.Copy, bias=MAGIC)
        nc.scalar.activation(rnd0[:rows, :], rnd0[:rows, :], AF.Copy, bias=-MAGIC)
        psi = dpool.tile([P, KC], f32, tag="psi")
        nc.vector.scalar_tensor_tensor(
            psi[:rows, :],
            in0=rnd0[:rows, :],
            scalar=-1.0,
            in1=a1[:rows, :],
            op0=OP.mult,
            op1=OP.add,
        )
        cmat = dpool.tile([P, KC], f32, tag="cmat")
        nc.scalar.activation(
            cmat[:rows, :], psi[:rows, :], AF.Sin, scale=2.0 * math.pi
        )
        nc.tensor.matmul(
            c_ps[:],
            gl[:rows, t : t + 1],
            cmat[:rows, :],
            start=(t == 0),
            stop=(t == N_JTILES - 1),
            skip_group_check=True,
        )

    csb = wpool.tile([1, KC], f32, tag=f"csb{ci}")
    nc.vector.tensor_copy(csb[:], c_ps[:])
    ck2 = wpool.tile([1, KC], f32, tag=f"ck2{ci}")
    nc.vector.tensor_tensor(ck2[:], csb[:], csb[:], op=OP.mult)
    nc.vector.tensor_tensor(ck2[:], ck2[:], kvec[:], op=OP.mult)
    s2 = wpool.tile([1, 1], f32, tag=f"s2{ci}")
    nc.vector.tensor_reduce(s2[:], ck2[:], axis=mybir.AxisListType.X, op=OP.add)
    # logdet = T*c0 + s2
    ld = wpool.tile([1, 1], f32, tag=f"ld{ci}")
    ld_op = nc.vector.scalar_tensor_tensor(
        ld[:], in0=csb[:, 0:1], scalar=float(T), in1=s2[:], op0=OP.mult, op1=OP.add
    )

    # ---------------- polynomial solve (phase 2) ----------------
    va = wpool.tile([P, NBLK + 2 * NPAD], f32, tag=f"va{ci}")
    vb = wpool.tile([P, NBLK + 2 * NPAD], f32, tag=f"vb{ci}")
    xs = wpool.tile([P, NBLK + 2 * NPAD], f32, tag=f"xs{ci}")
    gate_ops = [
        nc.vector.memset(va[:], 0.0),
        nc.vector.memset(vb[:], 0.0),
        nc.vector.memset(xs[:], 0.0),
    ]
    W_ps = ppool.tile([P, NBLK], f32, tag="W_ps")

    def matvec(dst_ps, src, mats):
        for m in range(5):
            off = m - 2
            nc.tensor.matmul(
                dst_ps[:],
                mats[:, m, :],
                src[:, NPAD + off : NPAD + off + NBLK],
                start=(m == 0),
                stop=(m == 4),
                skip_group_check=True,
            )

    # v0 = y; x = gamma_0 * y
    gate_ops.append(nc.vector.tensor_copy(va[:, NPAD : NPAD + NBLK], ysb[:]))
    gate_ops.append(
        nc.vector.tensor_scalar(
            xs[:, NPAD : NPAD + NBLK], ysb[:], float(gam[0]), None, op0=OP.mult
        )
    )
    # phase separation: the szego path owns DVE/ACT until ld is done;
    # interleaving its big DVE ops into the solve's latency-critical
    # PE->DVE->PE loop was measured to cost ~40 us.
    for op in gate_ops:
        add_dep_helper(op.ins, ld_op.ins, sync=True, reason="phase-separation")

    # v1 = As y = 0.5 * (2As) v0
    matvec(W_ps, va, NS2)
    nc.vector.tensor_scalar(
        vb[:, NPAD : NPAD + NBLK], W_ps[:], 0.5, None, op0=OP.mult
    )
    nc.vector.scalar_tensor_tensor(
        xs[:, NPAD : NPAD + NBLK],
        in0=vb[:, NPAD : NPAD + NBLK],
        scalar=float(gam[1]),
        in1=xs[:, NPAD : NPAD + NBLK],
        op0=OP.mult,
        op1=OP.add,
    )

    vold, vcur = va, vb
    for m in range(2, n_deg):
        matvec(W_ps, vcur, NS2)
        # v_new = W - v_old   (into v_old's buffer)
        nc.vector.scalar_tensor_tensor(
            vold[:, NPAD : NPAD + NBLK],
            in0=W_ps[:],
            scalar=1.0,
            in1=vold[:, NPAD : NPAD + NBLK],
            op0=OP.mult,
            op1=OP.subtract,
        )
        vold, vcur = vcur, vold
        # x += gamma_m * v_new  (off the critical path)
        nc.vector.scalar_tensor_tensor(
            xs[:, NPAD : NPAD + NBLK],
            in0=vcur[:, NPAD : NPAD + NBLK],
            scalar=float(gam[m]),
            in1=xs[:, NPAD : NPAD + NBLK],
            op0=OP.mult,
            op1=OP.add,
        )

    # quad = x^T (2y - A x)
    mv_ps = ppool.tile([P, NBLK], f32, tag="mv_ps")
    matvec(mv_ps, xs, NS)  # mv = -A x
    y2 = wpool.tile([P, NBLK], f32, tag=f"y2{ci}")
    nc.vector.tensor_scalar(y2[:], ysb[:], 2.0, None, op0=OP.mult)
    g2 = wpool.tile([P, NBLK], f32, tag=f"g2{ci}")
    nc.vector.scalar_tensor_tensor(
        g2[:], in0=mv_ps[:], scalar=1.0, in1=y2[:], op0=OP.mult, op1=OP.add
    )
    tq = wpool.tile([P, NBLK], f32, tag=f"tq{ci}")
    nc.vector.tensor_tensor(tq[:], xs[:, NPAD : NPAD + NBLK], g2[:], op=OP.mult)
    tred = wpool.tile([P, 1], f32, tag=f"tred{ci}")
    nc.vector.tensor_reduce(tred[:], tq[:], axis=mybir.AxisListType.X, op=OP.add)
    ones = cpool.tile([P, 1], f32, tag=f"ones{ci}")
    nc.vector.memset(ones[:], 1.0)
    quad_ps = ppool.tile([1, 1], f32, tag="quad_ps")
    nc.tensor.matmul(
        quad_ps[:], tred[:], ones[:], start=True, stop=True, skip_group_check=True
    )

    # out = -0.5*(quad + logdet)
    fin = wpool.tile([1, 1], f32, tag=f"fin{ci}")
    nc.vector.scalar_tensor_tensor(
        fin[:], in0=quad_ps[:], scalar=1.0, in1=ld[:], op0=OP.mult, op1=OP.add
    )
    nc.vector.tensor_scalar(fin[:], fin[:], -0.5, None, op0=OP.mult)
    nc.sync.dma_start(out_dram[:, ci : ci + 1], fin[:])

    if dbg_c is not None:
        nc.sync.dma_start(dbg_c[:], csb[:])
        nc.sync.dma_start(dbg_x[:], xs[:, NPAD : NPAD + NBLK])
        dq = wpool.tile([1, 4], f32, tag="dq")
        nc.vector.tensor_copy(dq[:, 0:1], quad_ps[:])
        nc.vector.tensor_copy(dq[:, 1:2], ld[:])
        nc.vector.tensor_copy(dq[:, 2:3], s2[:])
        nc.vector.tensor_copy(dq[:, 3:4], csb[:, 0:1])
        nc.sync.dma_start(dbg_ql[:], dq[:])


def _params_array(sig2, ell, var):
    row = np.array(
        [
            -1.0 / (2.0 * ell * ell),
            -var,
            -sig2,
            sig2,
            var * ell * math.sqrt(2.0 * math.pi),
            -(ell * ell) / 2.0,
            0.0,
            0.0,
        ],
        dtype=np.float32,
    )
    return np.tile(row[None, :], (P, 1))


def get_program(sig2, ell, var, n_deg=N_DEG, debug=False, n_copies=1, loop_n=0):
    key = (float(sig2), float(ell), float(var), int(n_deg), bool(debug), n_copies,
           loop_n)
    if key not in _prog_cache:
        _prog_cache[key] = _build(
            *key[:4], debug=key[4], n_copies=key[5], loop_n=key[6]
        )
    return _prog_cache[key]


def kernel(y, sigma_sq, lengthscale, variance):
    from concourse import bass_utils

    y = np.ascontiguousarray(np.asarray(y, dtype=np.float32))
    sig2 = float(np.asarray(sigma_sq).reshape(-1)[0])
    ell = float(np.asarray(lengthscale))
    var = float(np.asarray(variance))
    assert y.shape == (T,)

    nc = get_program(sig2, ell, var)
    par = _params_array(sig2, ell, var)
    in_map = {"y": y, "par": par}
    res = bass_utils.run_bass_kernel_spmd(
        nc, [dict(in_map) for _ in range(8)], core_ids=list(range(8))
    )
    out = res.results[0]["out"]
    return np.asarray(out, dtype=np.float32).reshape(1, 1)


if __name__ == "__main__":
    rng = np.random.default_rng(0)
    y = rng.standard_normal(T).astype(np.float32)
    o = kernel(y, np.ones(1, np.float32), np.float32(32.0), np.float32(1.0))
    print("kernel out:", o)



# revision 2
# speedup vs baseline: 1.2620x; 1.2620x over previous
"""GP marginal log-likelihood kernel for Trainium2 (Bass/Tile).

Computes -0.5 * y^T A^-1 y - 0.5 * logdet(A) for A = K + sigma^2 I where
K is the RBF covariance on the integer grid 0..T-1 (T=8192).

A is symmetric positive-definite *Toeplitz* and effectively *banded*
(entries vanish below f32 eps for |i-j| > 255 at lengthscale 32), and is
well conditioned: eig(A) in [sigma^2, sigma^2 + v*sum_d k(d)] (~[1, 81.2]).
This kernel exploits that structure instead of a dense 8192^3 factorization:

  * quad = y^T A^-1 y: x = p(A) y where p is a least-squares-optimal
    polynomial fitted on the host (from the hyperparameters alone) to the
    model spectrum of A (the symbol samples), expressed in the Chebyshev
    *second-kind* basis U_m so the device recurrence is the uniform
    three-term form with no special first step:
        q_0 = y,  q_1 = (2As) q_0,  q_{m+1} = (2As) q_m - q_{m-1},
        x = sum_m gamma_m q_m,
    where each (2As) q is a block-pentadiagonal matvec: 5 tensor-engine
    matmuls with 128x128 stationary band blocks.  The recurrence runs in a
    *hardware For_i loop* (two steps per trip, gamma read via loop-indexed
    scalar APs), so the emitted instruction stream is tiny.  The
    second-order functional quad = x^T (2y - A x) (with A x recovered from
    the same 2As operator: A x = ((2As)x - sh2 x)/sc2) makes the final
    error quadratic in the solver error (~4e-6 relative at degree 19).
  * logdet via the strong Szego limit theorem:
        logdet A = T*c_0 + sum_{k>=1} k*c_k^2,   c_k = Fourier coeffs of
    log f, f = the symbol of A.  T*c_0 is computed on device as the
    periodic-trapezoid mean of log f over a symmetric 512-point grid in
    [-pi, pi) (spectrally accurate; single Gaussian image suffices there).
    The small correction sum_k k*c_k^2 (~1.8 here) depends only on the
    scalar hyperparameters and is folded into the final combine constant
    on the host, like the polynomial coefficient schedule.

Everything data(y)-dependent runs on the device.  The host computes only
the iteration coefficient schedule and scalar constants from the scalar
hyperparameters (sigma^2, lengthscale, variance).  All 8 cores run the
same program on replicated inputs (the answer is a single scalar; core 0's
result is gathered).
"""

import math

import numpy as np

T = 8192
P = 128  # partitions
NBLK = T // P  # 64 column blocks
NPAD = 2  # zero pad columns on each side of the padded vec tiles
N_GRID = 512  # Szego c0 quadrature grid (symmetric, full circle)
NGC = N_GRID // P  # 4 grid columns
N_DEG = 19  # number of polynomial coefficients (highest index 18, even)
NTRIP = (N_DEG - 1) // 2  # hardware-loop trips, 2 recurrence steps each
GW = 16  # gamma table half-width (max NTRIP supported)

_prog_cache = {}


def _ls_poly_U(sig2, ell, var, n_deg):
    """Host-side schedule: LS-optimal solve polynomial in the U basis.

    Fits p(lam) = sum_m gamma_m U_m(s(lam)) minimizing
    sum_j (1 - lam_j p(lam_j))^2 / lam_j over the model spectrum
    lam_j = f(2 pi j / T) (symbol samples, the asymptotic eigenvalue
    distribution of A).  Returns (gamma, lo, hi)."""
    th = np.linspace(0.0, np.pi, T // 2 + 1)
    lam = sig2 + var * ell * math.sqrt(2.0 * math.pi) * (
        np.exp(-((ell * th) ** 2) / 2.0)
        + np.exp(-((ell * (th - 2 * math.pi)) ** 2) / 2.0)
    )
    lo, hi = float(lam.min()), float(lam.max())
    xs = (2.0 * lam - (hi + lo)) / (hi - lo)
    V = np.zeros((lam.size, n_deg))
    V[:, 0] = 1.0
    if n_deg > 1:
        V[:, 1] = 2.0 * xs  # U_1 = 2x
    for m in range(2, n_deg):
        V[:, m] = 2.0 * xs * V[:, m - 1] - V[:, m - 2]
    w = 1.0 / lam
    Aw = V * (lam * np.sqrt(w))[:, None]
    b = np.sqrt(w)
    g, *_ = np.linalg.lstsq(Aw, b, rcond=None)
    return g, lo, hi


def _szego_corr(sig2, ell, var):
    """Host-side scalar: sum_{k>=1} k c_k^2 for the symbol of A (pure
    function of the hyperparameters, like the gamma schedule)."""
    N = 65536
    th = 2.0 * np.pi * np.arange(N) / N
    s = np.zeros(N)
    for m in (-2, -1, 0, 1, 2):
        s += np.exp(-((ell * (th - 2.0 * np.pi * m)) ** 2) / 2.0)
    f = sig2 + var * ell * math.sqrt(2.0 * math.pi) * s
    ck = np.fft.rfft(np.log(f)).real / N
    k = np.arange(1, 4097)
    return float(np.sum(k * ck[1:4097] ** 2))


def _gtab_array(sig2, ell, var):
    """[P, 2*GW] gamma table: col j = gamma_{2j+1}, col GW+j = gamma_{2j+2}."""
    gam, _, _ = _ls_poly_U(sig2, ell, var, N_DEG)
    row = np.zeros(2 * GW, dtype=np.float32)
    for j in range(GW):
        if 2 * j + 1 < N_DEG:
            row[j] = gam[2 * j + 1]
        if 2 * j + 2 < N_DEG:
            row[GW + j] = gam[2 * j + 2]
    return np.tile(row[None, :], (P, 1))


def make_in_map(sig2, ell, var, y):
    return {
        "y": np.ascontiguousarray(np.asarray(y, dtype=np.float32)),
        "gtab": _gtab_array(sig2, ell, var),
    }


def _build(sig2, ell, var, n_copies=1, debug=False):
    """Emit the full program into a fresh Bacc instance and return it."""
    import concourse.mybir as mybir
    import concourse.tile as tile
    from concourse import bacc

    f32 = mybir.dt.float32

    nc = bacc.Bacc("TRN2", target_bir_lowering=False, debug=False)
    y_dram = nc.dram_tensor("y", [T], f32, kind="ExternalInput")
    gtab_dram = nc.dram_tensor("gtab", [P, 2 * GW], f32, kind="ExternalInput")
    out_dram = nc.dram_tensor("out", [1, n_copies], f32, kind="ExternalOutput")
    dbg = (
        nc.dram_tensor("dbg", [P, NBLK + 1], f32, kind="ExternalOutput")
        if debug
        else None
    )

    with tile.TileContext(nc) as tc:
        with (
            tc.tile_pool(name="const", bufs=1) as cpool,
            tc.tile_pool(name="work", bufs=1) as wpool,
            tc.tile_pool(name="ps", bufs=1, space="PSUM") as ppool,
        ):
            for ci in range(n_copies):
                _emit_one(
                    nc, tc, cpool, wpool, ppool, mybir,
                    y_dram, gtab_dram, out_dram, dbg if ci == 0 else None,
                    sig2, ell, var, ci,
                )

    nc.compile()
    return nc


def _emit_one(
    nc, tc, cpool, wpool, ppool, mybir,
    y_dram, gtab_dram, out_dram, dbg,
    sig2, ell, var, ci,
):
    from concourse.bass import ds

    f32 = mybir.dt.float32
    AF = mybir.ActivationFunctionType
    OP = mybir.AluOpType

    gam, lam_lo, lam_hi = _ls_poly_U(sig2, ell, var, N_DEG)
    # 2*As = sc2*A + sh2*I
    sc2 = 4.0 / (lam_hi - lam_lo)
    sh2 = -2.0 * (lam_hi + lam_lo) / (lam_hi - lam_lo)
    corr = _szego_corr(sig2, ell, var)

    gtab = cpool.tile([P, 2 * GW], f32, tag=f"gtab{ci}")
    nc.sync.dma_start(gtab[:], gtab_dram[:])
    godd = gtab[:, 0:GW]
    gevn = gtab[:, GW : 2 * GW]

    # padded vector tiles; window cols NPAD..NPAD+NBLK hold blocks 0..63.
    # y lands in the window via a strided DMA (partition = intra-block idx).
    yt = y_dram.rearrange("(b r) -> r b", b=NBLK)
    va = wpool.tile([P, NBLK + 2 * NPAD], f32, tag=f"va{ci}")  # y, preserved
    nc.vector.memset(va[:], 0.0)
    nc.sync.dma_start(va[:, NPAD : NPAD + NBLK], yt)
    vb = wpool.tile([P, NBLK + 2 * NPAD], f32, tag=f"vb{ci}")  # q_even chain
    nc.vector.memset(vb[:], 0.0)
    nc.sync.dma_start(vb[:, NPAD : NPAD + NBLK], yt)
    vc = wpool.tile([P, NBLK + 2 * NPAD], f32, tag=f"vc{ci}")  # q_odd chain
    nc.vector.memset(vc[:], 0.0)
    xs = wpool.tile([P, NBLK + 2 * NPAD], f32, tag=f"xs{ci}")
    nc.vector.memset(xs[:], 0.0)
    nc.vector.tensor_scalar(
        xs[:, NPAD : NPAD + NBLK], va[:, NPAD : NPAD + NBLK],
        float(gam[0]), None, op0=OP.mult,
    )

    # ---------------- band block matrices (2As operator) ----------------
    # NS2[c, m, r] = sc2*var*exp(-(c-r+128(m-2))^2/(2 l^2)) off-diagonal,
    # diagonal = sc2*(var+sig2) + sh2 (constant), via affine_select fill.
    NS2 = cpool.tile([P, 5, P], f32, tag=f"NS2{ci}")
    nc.gpsimd.iota(
        NS2[:], pattern=[[P, 5], [-1, P]], base=-2 * P, channel_multiplier=1,
        allow_small_or_imprecise_dtypes=True,
    )
    nc.scalar.activation(NS2[:], NS2[:], AF.Square)
    nc.scalar.activation(
        NS2[:], NS2[:], AF.Exp, scale=float(-1.0 / (2.0 * ell * ell))
    )
    nc.vector.tensor_scalar(
        NS2[:], NS2[:], float(sc2 * var), None, op0=OP.mult
    )
    nc.gpsimd.affine_select(
        out=NS2[:, 2, :], in_=NS2[:, 2, :],
        compare_op=mybir.AluOpType.not_equal,
        fill=float(sc2 * (var + sig2) + sh2),
        base=0, pattern=[[-1, P]], channel_multiplier=1,
    )

    # ---------------- Szego logdet: T*c0 by periodic trapezoid ----------
    # grid theta = 2 pi j / N, j = -256..255 (symmetric: one Gaussian image)
    pair = wpool.tile([P, 2], f32, tag=f"pair{ci}")
    gl = wpool.tile([P, NGC], f32, tag=f"gl{ci}")
    nc.gpsimd.iota(
        gl[:], pattern=[[P, NGC]], base=-N_GRID // 2, channel_multiplier=1,
        allow_small_or_imprecise_dtypes=True,
    )
    nc.scalar.activation(
        gl[:], gl[:], AF.Square, scale=float(2.0 * math.pi / N_GRID)
    )
    nc.scalar.activation(
        gl[:], gl[:], AF.Exp, scale=float(-(ell * ell) / 2.0)
    )
    # ln(v*l*sqrt(2pi) * e + sig2)
    nc.scalar.activation(
        gl[:], gl[:], AF.Ln,
        scale=float(var * ell * math.sqrt(2.0 * math.pi)), bias=float(sig2),
    )
    nc.vector.tensor_reduce(
        pair[:, 1:2], gl[:], axis=mybir.AxisListType.X, op=OP.add
    )

    # ---------------- polynomial solve: hardware loop --------------------
    W_ps = ppool.tile([P, NBLK], f32, tag="W_ps")

    def matvec(dst_ps, src):
        for m in range(5):
            nc.tensor.matmul(
                dst_ps[:],
                NS2[:, m, :],
                src[:, m : m + NBLK],
                start=(m == 0),
                stop=(m == 4),
                skip_group_check=True,
            )

    win = slice(NPAD, NPAD + NBLK)
    with tc.For_i(0, NTRIP, 1) as i:
        # odd step: vc = (2As) vb - vc;  xs += gamma_{2i+1} vc
        matvec(W_ps, vb)
        nc.vector.scalar_tensor_tensor(
            vc[:, win], in0=W_ps[:], scalar=1.0, in1=vc[:, win],
            op0=OP.mult, op1=OP.subtract,
        )
        nc.vector.scalar_tensor_tensor(
            xs[:, win], in0=vc[:, win], scalar=godd[:, ds(i, 1)], in1=xs[:, win],
            op0=OP.mult, op1=OP.add,
        )
        # even step: vb = (2As) vc - vb;  xs += gamma_{2i+2} vb
        matvec(W_ps, vc)
        nc.vector.scalar_tensor_tensor(
            vb[:, win], in0=W_ps[:], scalar=1.0, in1=vb[:, win],
            op0=OP.mult, op1=OP.subtract,
        )
        nc.vector.scalar_tensor_tensor(
            xs[:, win], in0=vb[:, win], scalar=gevn[:, ds(i, 1)], in1=xs[:, win],
            op0=OP.mult, op1=OP.add,
        )

    # ---------------- quad = x^T (2y - A x), A x from the 2As operator ---
    # r = y - A x = (sh2/sc2) x + y - (1/sc2) (2As) x;  quad = x^T (y + r)
    matvec(W_ps, xs)
    t0 = wpool.tile([P, NBLK], f32, tag=f"t0{ci}")
    nc.vector.scalar_tensor_tensor(
        t0[:], in0=xs[:, win], scalar=float(sh2 / sc2), in1=va[:, win],
        op0=OP.mult, op1=OP.add,
    )
    nc.vector.scalar_tensor_tensor(
        t0[:], in0=W_ps[:], scalar=float(-1.0 / sc2), in1=t0[:],
        op0=OP.mult, op1=OP.add,
    )
    nc.vector.tensor_tensor(t0[:], t0[:], va[:, win], op=OP.add)  # y + r
    nc.vector.tensor_tensor(t0[:], t0[:], xs[:, win], op=OP.mult)
    nc.vector.tensor_reduce(
        pair[:, 0:1], t0[:], axis=mybir.AxisListType.X, op=OP.add
    )

    # partition-reduce both sums in one matmul: out_ps[0, k] = sum_p pair[p, k]
    ones = cpool.tile([P, 1], f32, tag=f"ones{ci}")
    nc.vector.memset(ones[:], 1.0)
    out_ps = ppool.tile([1, 2], f32, tag="out_ps")
    nc.tensor.matmul(
        out_ps[:], ones[:], pair[:], start=True, stop=True,
        skip_group_check=True,
    )

    # out = -0.5*quad - 0.5*((T/N)*sum(log f) + corr)
    fin = wpool.tile([1, 2], f32, tag=f"fin{ci}")
    nc.vector.tensor_scalar(
        fin[:, 1:2], out_ps[:, 1:2],
        float(-0.5 * T / N_GRID), float(-0.5 * corr),
        op0=OP.mult, op1=OP.add,
    )
    nc.vector.scalar_tensor_tensor(
        fin[:, 0:1], in0=out_ps[:, 0:1], scalar=-0.5, in1=fin[:, 1:2],
        op0=OP.mult, op1=OP.add,
    )
    nc.sync.dma_start(out_dram[:, ci : ci + 1], fin[:, 0:1])

    if dbg is not None:
        dq = wpool.tile([P, NBLK + 1], f32, tag="dq")
        nc.vector.tensor_copy(dq[:, :NBLK], xs[:, win])
        nc.vector.tensor_copy(dq[:1, NBLK : NBLK + 1], pair[:1, 1:2])
        nc.sync.dma_start(dbg[:], dq[:])


def get_program(sig2, ell, var, n_copies=1, debug=False):
    key = (float(sig2), float(ell), float(var), int(n_copies), bool(debug))
    if key not in _prog_cache:
        _prog_cache[key] = _build(
            *key[:3], n_copies=key[3], debug=key[4]
        )
    return _prog_cache[key]


def kernel(y, sigma_sq, lengthscale, variance):
    from concourse import bass_utils

    y = np.ascontiguousarray(np.asarray(y, dtype=np.float32))
    sig2 = float(np.asarray(sigma_sq).reshape(-1)[0])
    ell = float(np.asarray(lengthscale))
    var = float(np.asarray(variance))
    assert y.shape == (T,)

    nc = get_program(sig2, ell, var)
    in_map = make_in_map(sig2, ell, var, y)
    res = bass_utils.run_bass_kernel_spmd(
        nc, [dict(in_map) for _ in range(8)], core_ids=list(range(8))
    )
    out = res.results[0]["out"]
    return np.asarray(out, dtype=np.float32).reshape(1, 1)


if __name__ == "__main__":
    rng = np.random.default_rng(0)
    y = rng.standard_normal(T).astype(np.float32)
    o = kernel(y, np.ones(1, np.float32), np.float32(32.0), np.float32(1.0))
    print("kernel out:", o)


# revision 12
# speedup vs baseline: 3.4230x; 2.7123x over previous
"""GP marginal log-likelihood kernel for Trainium2 (Bass/Tile).

Computes -0.5 * y^T A^-1 y - 0.5 * logdet(A) for A = K + sigma^2 I where
K is the RBF covariance on the integer grid 0..T-1 (T=8192).

A is symmetric positive-definite *Toeplitz* and effectively *banded*
(entries vanish below f32 eps for |i-j| > 255 at lengthscale 32), and is
well conditioned: eig(A) in [sigma^2, sigma^2 + v*sum_d k(d)] (~[1, 81.2]).
This kernel exploits that structure instead of a dense 8192^3 factorization:

  * quad = y^T A^-1 y: x = p(A) y where p is a least-squares-optimal
    polynomial fitted on the host (from the hyperparameters alone) to the
    model spectrum of A (the symbol samples), expressed in the Chebyshev
    *second-kind* basis U_m so the device recurrence is the uniform
    three-term form with no special first step:
        q_0 = y,  q_{m+1} = (2As) q_m - q_{m-1}  (q_{-1} = 0),
        x = sum_m gamma_m q_m,
    where each (2As) q is a block-tridiagonal matvec: 3 tensor-engine
    matmuls with 128x128 stationary band blocks (the |i-j| in [129,255]
    tail that falls outside +-1 block reach is ~3e-4 and verified
    numerically to not matter at the required tolerance).  The recurrence
    runs in a *hardware For_i loop*, one step per trip: the q_m live in
    one big SBUF tile as 66-column padded slots addressed by the loop
    variable, and gamma_m comes from a loop-indexed scalar AP.  The
    second-order functional quad = x^T (2y - A x) (with A x recovered
    from the same 2As operator: A x = ((2As)x - sh2 x)/sc2) makes the
    final error quadratic in the solver error (~5e-6 relative, degree 18).
  * logdet via the strong Szego limit theorem:
        logdet A = T*c_0 + sum_{k>=1} k*c_k^2,   c_k = Fourier coeffs of
    log f, f = the symbol of A.  T*c_0 is computed on device as the
    periodic-trapezoid mean of log f over a symmetric 512-point grid in
    [-pi, pi) (spectrally accurate; single Gaussian image suffices there).
    The small correction sum_k k*c_k^2 (~1.8 here) depends only on the
    scalar hyperparameters and is folded into the final combine constant
    on the host, like the polynomial coefficient schedule.

Everything data(y)-dependent runs on the device.  The host computes only
the iteration coefficient schedule and scalar constants from the scalar
hyperparameters (sigma^2, lengthscale, variance).  All 8 cores run the
same program on replicated inputs (the answer is a single scalar; core 0's
result is gathered).
"""

import math

import numpy as np

T = 8192
P = 128  # partitions
NBLK = T // P  # 64 column blocks
SW = NBLK + 2  # 66: one pad column each side of a 64-block slot
N_GRID = 512  # Szego c0 quadrature grid (symmetric, full circle)
NGC = N_GRID // P  # 4 grid columns
N_DEG = 19  # number of polynomial coefficients (highest index 18)
NTRIP = N_DEG - 1  # hardware-loop trips, one recurrence step each
NSLOT = N_DEG + 3  # q_{-1}(=0), q_0..q_{N_DEG}, x accumulator
GW = 32  # gamma table width

_prog_cache = {}


def _ls_poly_U(sig2, ell, var, n_deg):
    """Host-side schedule: LS-optimal solve polynomial in the U basis.

    Fits p(lam) = sum_m gamma_m U_m(s(lam)) minimizing
    sum_j (1 - lam_j p(lam_j))^2 / lam_j over the model spectrum
    lam_j = f(2 pi j / T) (symbol samples, the asymptotic eigenvalue
    distribution of A).  Returns (gamma, lo, hi)."""
    th = np.linspace(0.0, np.pi, T // 2 + 1)
    lam = sig2 + var * ell * math.sqrt(2.0 * math.pi) * (
        np.exp(-((ell * th) ** 2) / 2.0)
        + np.exp(-((ell * (th - 2 * math.pi)) ** 2) / 2.0)
    )
    lo, hi = float(lam.min()), float(lam.max())
    xs = (2.0 * lam - (hi + lo)) / (hi - lo)
    V = np.zeros((lam.size, n_deg))
    V[:, 0] = 1.0
    if n_deg > 1:
        V[:, 1] = 2.0 * xs  # U_1 = 2x
    for m in range(2, n_deg):
        V[:, m] = 2.0 * xs * V[:, m - 1] - V[:, m - 2]
    w = 1.0 / lam
    Aw = V * (lam * np.sqrt(w))[:, None]
    b = np.sqrt(w)
    g, *_ = np.linalg.lstsq(Aw, b, rcond=None)
    return g, lo, hi


def _szego_corr(sig2, ell, var):
    """Host-side scalar: sum_{k>=1} k c_k^2 for the symbol of A (pure
    function of the hyperparameters, like the gamma schedule)."""
    N = 65536
    th = 2.0 * np.pi * np.arange(N) / N
    s = np.zeros(N)
    for m in (-2, -1, 0, 1, 2):
        s += np.exp(-((ell * (th - 2.0 * np.pi * m)) ** 2) / 2.0)
    f = sig2 + var * ell * math.sqrt(2.0 * math.pi) * s
    ck = np.fft.rfft(np.log(f)).real / N
    k = np.arange(1, 4097)
    return float(np.sum(k * ck[1:4097] ** 2))


def _gtab_array(sig2, ell, var):
    """[P, GW] table: col j = gamma_j (j < N_DEG); col GW-1 = 1.0
    (the all-ones stationary column for the partition-sum matmul)."""
    gam, _, _ = _ls_poly_U(sig2, ell, var, N_DEG)
    row = np.zeros(GW, dtype=np.float32)
    row[:N_DEG] = gam
    row[GW - 1] = 1.0
    return np.tile(row[None, :], (P, 1))


DEFAULT_VARIANT = "U9"


def make_in_map(sig2, ell, var, y, variant=DEFAULT_VARIANT):
    im = {"y": np.ascontiguousarray(np.asarray(y, dtype=np.float32))}
    if not variant.startswith("U"):
        im["gtab"] = _gtab_array(sig2, ell, var)
    return im


def _build(sig2, ell, var, n_copies=1, debug=False, variant="A"):
    """Emit the full program into a fresh Bacc instance and return it."""
    import concourse.mybir as mybir
    import concourse.tile as tile
    from concourse import bacc

    f32 = mybir.dt.float32

    nc = bacc.Bacc("TRN2", target_bir_lowering=False, debug=False)
    y_dram = nc.dram_tensor("y", [T], f32, kind="ExternalInput")
    gtab_dram = (
        None
        if variant.startswith("U")
        else nc.dram_tensor("gtab", [P, GW], f32, kind="ExternalInput")
    )
    out_dram = nc.dram_tensor("out", [1, n_copies], f32, kind="ExternalOutput")
    dbg = (
        nc.dram_tensor("dbg", [P, NBLK + 1], f32, kind="ExternalOutput")
        if debug
        else None
    )

    with tile.TileContext(nc) as tc:
        with (
            tc.tile_pool(name="const", bufs=1) as cpool,
            tc.tile_pool(name="work", bufs=1) as wpool,
            tc.tile_pool(name="ps", bufs=1, space="PSUM") as ppool,
        ):
            emit = (_emit_one_u if variant.startswith("U")
                    else _emit_one if variant in ("A", "D") else _emit_one_b)
            for ci in range(n_copies):
                emit(
                    nc, tc, cpool, wpool, ppool, mybir,
                    y_dram, gtab_dram, out_dram, dbg if ci == 0 else None,
                    sig2, ell, var, ci, variant,
                )

    nc.compile()
    return nc


def _emit_one(
    nc, tc, cpool, wpool, ppool, mybir,
    y_dram, gtab_dram, out_dram, dbg,
    sig2, ell, var, ci, variant="A",
):
    """Variant A: hardware For_i loop, one recurrence step per trip.
    Trip i computes q_{i+1} = (2As) q_i - q_{i-1} into slot i+2 and
    accumulates x += gamma_i q_i (slot i+1), so no xs-init op is needed.
    Szego grid recycled from the squared NS2 iota; single merged reduce."""
    from concourse.bass import ds

    f32 = mybir.dt.float32
    AF = mybir.ActivationFunctionType
    OP = mybir.AluOpType

    gam, lam_lo, lam_hi = _ls_poly_U(sig2, ell, var, N_DEG)
    sc2 = 4.0 / (lam_hi - lam_lo)
    sh2 = -2.0 * (lam_hi + lam_lo) / (lam_hi - lam_lo)
    corr = _szego_corr(sig2, ell, var) + _szego_missing(sig2, ell, var)

    gtab = cpool.tile([P, GW], f32, tag=f"gtab{ci}")
    nc.sync.dma_start(gtab[:], gtab_dram[:])

    # slots: 0 = q_{-1} (zero), 1 = q_0 = y, ..., NSLOT-2 = q_{N_DEG}
    # (extra, unused), NSLOT-1 = x accumulator
    big = wpool.tile([P, NSLOT * SW], f32, tag=f"big{ci}")
    nc.vector.memset(big[:], 0.0)
    yw = big[:, SW + 1 : SW + 1 + NBLK]
    nc.sync.dma_start(yw, y_dram.rearrange("(b r) -> r b", b=NBLK))
    XO = (NSLOT - 1) * SW
    xt = big[:, XO : XO + SW]
    xw = big[:, XO + 1 : XO + 1 + NBLK]

    NS2 = cpool.tile([P, 3, P], f32, tag=f"NS2{ci}")
    nc.gpsimd.iota(
        NS2[:], pattern=[[P, 3], [-1, P]], base=-P, channel_multiplier=1,
        allow_small_or_imprecise_dtypes=True,
    )
    nc.scalar.activation(NS2[:], NS2[:], AF.Square)
    t0 = wpool.tile([P, NBLK + 3], f32, tag=f"t0{ci}")
    gl = t0[:, NBLK : NBLK + 3]
    th_sc = (2.0 * math.pi / N_GRID) ** 2 * ell * ell / 2.0
    nc.scalar.activation(gl, NS2[:, :, 0], AF.Exp, scale=float(-th_sc))
    nc.scalar.activation(
        gl, gl, AF.Ln,
        scale=float(var * ell * math.sqrt(2.0 * math.pi)),
        bias=_bias_arg(nc, cpool, mybir, sig2, f"sgb{ci}"),
    )
    nc.vector.tensor_scalar(gl, gl, float(-0.5 * T / N_GRID), None, op0=OP.mult)
    nc.scalar.activation(
        NS2[:], NS2[:], AF.Exp, scale=float(-1.0 / (2.0 * ell * ell))
    )
    nc.vector.tensor_scalar(
        NS2[:], NS2[:], float(sc2 * var), None, op0=OP.mult
    )
    nc.gpsimd.affine_select(
        out=NS2[:, 1, :], in_=NS2[:, 1, :],
        compare_op=mybir.AluOpType.not_equal,
        fill=float(sc2 * (var + sig2) + sh2),
        base=0, pattern=[[-1, P]], channel_multiplier=1,
    )

    W_ps = ppool.tile([P, NBLK], f32, tag="W_ps")

    with tc.For_i(0, N_DEG, 1, staggered_reset=(variant == "D")) as i:
        for m in range(3):
            nc.tensor.matmul(
                W_ps[:],
                NS2[:, m, :],
                big[:, ds(i * SW + SW + m, NBLK)],
                start=(m == 0),
                stop=(m == 2),
                skip_group_check=True,
            )
        nc.vector.scalar_tensor_tensor(
            big[:, ds(i * SW + 2 * SW + 1, NBLK)],
            in0=W_ps[:], scalar=1.0, in1=big[:, ds(i * SW + 1, NBLK)],
            op0=OP.mult, op1=OP.subtract,
        )
        nc.vector.scalar_tensor_tensor(
            xw, in0=big[:, ds(i * SW + SW + 1, NBLK)],
            scalar=gtab[:, ds(i, 1)], in1=xw,
            op0=OP.mult, op1=OP.add,
        )

    # quad tail (same as U): t0[:, 0:64] = -0.5 * x * (2y - A x)
    for m in range(3):
        nc.tensor.matmul(
            W_ps[:], NS2[:, m, :], big[:, XO + m : XO + m + NBLK],
            start=(m == 0), stop=(m == 2), skip_group_check=True,
        )
    tq = t0[:, 0:NBLK]
    nc.vector.scalar_tensor_tensor(
        tq, in0=xw, scalar=float(sh2), in1=W_ps[:],
        op0=OP.mult, op1=OP.subtract,
    )
    nc.vector.scalar_tensor_tensor(
        tq, in0=tq, scalar=float(-0.5 / sc2), in1=yw,
        op0=OP.mult, op1=OP.subtract,
    )
    nc.vector.tensor_tensor(tq, tq, xw, op=OP.mult)

    red = wpool.tile([P, 1], f32, tag=f"red{ci}")
    nc.vector.tensor_reduce(red[:], t0[:], axis=mybir.AxisListType.X, op=OP.add)
    out_ps = ppool.tile([1, 1], f32, tag="out_ps")
    nc.tensor.matmul(
        out_ps[:], gtab[:, GW - 1 : GW], red[:], start=True, stop=True,
        skip_group_check=True,
    )
    fin = wpool.tile([1, 1], f32, tag=f"fin{ci}")
    nc.vector.tensor_scalar(
        fin[:], out_ps[:], 1.0, float(-0.5 * corr), op0=OP.mult, op1=OP.add
    )
    nc.sync.dma_start(out_dram[:, ci : ci + 1], fin[:])


def get_program(sig2, ell, var, n_copies=1, debug=False, variant=DEFAULT_VARIANT):
    key = (float(sig2), float(ell), float(var), int(n_copies), bool(debug), variant)
    if key not in _prog_cache:
        _prog_cache[key] = _build(
            *key[:3], n_copies=key[3], debug=key[4], variant=key[5]
        )
    return _prog_cache[key]


def kernel(y, sigma_sq, lengthscale, variance):
    from concourse import bass_utils

    y = np.ascontiguousarray(np.asarray(y, dtype=np.float32))
    sig2 = float(np.asarray(sigma_sq).reshape(-1)[0])
    ell = float(np.asarray(lengthscale))
    var = float(np.asarray(variance))
    assert y.shape == (T,)

    nc = get_program(sig2, ell, var)
    in_map = make_in_map(sig2, ell, var, y)

    res = bass_utils.run_bass_kernel_spmd(
        nc, [dict(in_map) for _ in range(8)], core_ids=list(range(8))
    )
    out = res.results[0]["out"]
    return np.asarray(out, dtype=np.float32).reshape(1, 1)


if __name__ == "__main__":
    rng = np.random.default_rng(0)
    y = rng.standard_normal(T).astype(np.float32)
    o = kernel(y, np.ones(1, np.float32), np.float32(32.0), np.float32(1.0))
    print("kernel out:", o)


def _emit_one_b(
    nc, tc, cpool, wpool, ppool, mybir,
    y_dram, gtab_dram, out_dram, dbg,
    sig2, ell, var, ci, variant="B",
):
    """Variant B: double-step For_i loop (NTRIP/2 trips), static matmul
    operands (vb/vc slots), ds() only on the gamma scalar APs."""
    from concourse.bass import ds

    f32 = mybir.dt.float32
    AF = mybir.ActivationFunctionType
    OP = mybir.AluOpType

    gam, lam_lo, lam_hi = _ls_poly_U(sig2, ell, var, N_DEG)
    sc2 = 4.0 / (lam_hi - lam_lo)
    sh2 = -2.0 * (lam_hi + lam_lo) / (lam_hi - lam_lo)
    corr = _szego_corr(sig2, ell, var)

    gtab = cpool.tile([P, GW], f32, tag=f"gtab{ci}")
    nc.sync.dma_start(gtab[:], gtab_dram[:])
    # B-layout gamma table: col j = gamma_{2j+1}, col 8+j = gamma_{2j+2}
    # (host array handled by _gtab_array_b); ones col GW-1.
    godd = gtab[:, 0:12]
    gevn = gtab[:, 12:24]

    big = wpool.tile([P, 4 * SW], f32, tag=f"big{ci}")
    nc.vector.memset(big[:], 0.0)
    vy = big[:, 0 * SW : 0 * SW + SW]
    vb = big[:, 1 * SW : 1 * SW + SW]
    vc = big[:, 2 * SW : 2 * SW + SW]
    xt = big[:, 3 * SW : 3 * SW + SW]
    yw = vy[:, 1 : 1 + NBLK]
    nc.sync.dma_start(yw, y_dram.rearrange("(b r) -> r b", b=NBLK))
    nc.sync.dma_start(vb[:, 1 : 1 + NBLK], y_dram.rearrange("(b r) -> r b", b=NBLK))
    xw = xt[:, 1 : 1 + NBLK]
    nc.vector.tensor_scalar(xw, yw, float(gam[0]), None, op0=OP.mult)

    NS2 = cpool.tile([P, 3, P], f32, tag=f"NS2{ci}")
    nc.gpsimd.iota(
        NS2[:], pattern=[[P, 3], [-1, P]], base=-P, channel_multiplier=1,
        allow_small_or_imprecise_dtypes=True,
    )
    nc.scalar.activation(NS2[:], NS2[:], AF.Square)
    nc.scalar.activation(
        NS2[:], NS2[:], AF.Exp, scale=float(-1.0 / (2.0 * ell * ell))
    )
    nc.vector.tensor_scalar(
        NS2[:], NS2[:], float(sc2 * var), None, op0=OP.mult
    )
    nc.gpsimd.affine_select(
        out=NS2[:, 1, :], in_=NS2[:, 1, :],
        compare_op=mybir.AluOpType.not_equal,
        fill=float(sc2 * (var + sig2) + sh2),
        base=0, pattern=[[-1, P]], channel_multiplier=1,
    )

    pair = wpool.tile([P, 2], f32, tag=f"pair{ci}")
    gl = wpool.tile([P, NGC], f32, tag=f"gl{ci}")
    nc.gpsimd.iota(
        gl[:], pattern=[[P, NGC]], base=-N_GRID // 2, channel_multiplier=1,
        allow_small_or_imprecise_dtypes=True,
    )
    nc.scalar.activation(
        gl[:], gl[:], AF.Square, scale=float(2.0 * math.pi / N_GRID)
    )
    nc.scalar.activation(
        gl[:], gl[:], AF.Exp, scale=float(-(ell * ell) / 2.0)
    )
    nc.scalar.activation(
        gl[:], gl[:], AF.Ln,
        scale=float(var * ell * math.sqrt(2.0 * math.pi)), bias=float(sig2),
    )
    nc.vector.tensor_reduce(
        pair[:, 1:2], gl[:], axis=mybir.AxisListType.X, op=OP.add
    )

    W_ps = ppool.tile([P, NBLK], f32, tag="W_ps")

    def matvec(src):
        for m in range(3):
            nc.tensor.matmul(
                W_ps[:], NS2[:, m, :], src[:, m : m + NBLK],
                start=(m == 0), stop=(m == 2), skip_group_check=True,
            )

    assert NTRIP % 2 == 0
    with tc.For_i(0, NTRIP // 2, 1) as i:
        matvec(vb)
        nc.vector.scalar_tensor_tensor(
            vc[:, 1 : 1 + NBLK], in0=W_ps[:], scalar=1.0,
            in1=vc[:, 1 : 1 + NBLK], op0=OP.mult, op1=OP.subtract,
        )
        nc.vector.scalar_tensor_tensor(
            xw, in0=vc[:, 1 : 1 + NBLK], scalar=godd[:, ds(i, 1)], in1=xw,
            op0=OP.mult, op1=OP.add,
        )
        matvec(vc)
        nc.vector.scalar_tensor_tensor(
            vb[:, 1 : 1 + NBLK], in0=W_ps[:], scalar=1.0,
            in1=vb[:, 1 : 1 + NBLK], op0=OP.mult, op1=OP.subtract,
        )
        nc.vector.scalar_tensor_tensor(
            xw, in0=vb[:, 1 : 1 + NBLK], scalar=gevn[:, ds(i, 1)], in1=xw,
            op0=OP.mult, op1=OP.add,
        )

    matvec(xt)
    t0 = wpool.tile([P, NBLK], f32, tag=f"t0{ci}")
    nc.vector.scalar_tensor_tensor(
        t0[:], in0=xw, scalar=float(sh2), in1=W_ps[:],
        op0=OP.mult, op1=OP.subtract,
    )
    nc.vector.scalar_tensor_tensor(
        t0[:], in0=t0[:], scalar=float(1.0 / sc2), in1=yw,
        op0=OP.mult, op1=OP.add,
    )
    nc.vector.tensor_tensor(t0[:], t0[:], yw, op=OP.add)
    nc.vector.tensor_tensor(t0[:], t0[:], xw, op=OP.mult)
    nc.vector.tensor_reduce(
        pair[:, 0:1], t0[:], axis=mybir.AxisListType.X, op=OP.add
    )

    out_ps = ppool.tile([1, 2], f32, tag="out_ps")
    nc.tensor.matmul(
        out_ps[:], gtab[:, GW - 1 : GW], pair[:], start=True, stop=True,
        skip_group_check=True,
    )
    fin = wpool.tile([1, 2], f32, tag=f"fin{ci}")
    nc.vector.tensor_scalar(
        fin[:, 1:2], out_ps[:, 1:2],
        float(-0.5 * T / N_GRID), float(-0.5 * corr),
        op0=OP.mult, op1=OP.add,
    )
    nc.vector.scalar_tensor_tensor(
        fin[:, 0:1], in0=out_ps[:, 0:1], scalar=-0.5, in1=fin[:, 1:2],
        op0=OP.mult, op1=OP.add,
    )
    nc.sync.dma_start(out_dram[:, ci : ci + 1], fin[:, 0:1])


def _gtab_array_b(sig2, ell, var):
    gam, _, _ = _ls_poly_U(sig2, ell, var, N_DEG)
    row = np.zeros(GW, dtype=np.float32)
    for j in range(12):
        if 2 * j + 1 < N_DEG:
            row[j] = gam[2 * j + 1]
        if 2 * j + 2 < N_DEG:
            row[12 + j] = gam[2 * j + 2]
    row[GW - 1] = 1.0
    return np.tile(row[None, :], (P, 1))


def _szego_missing(sig2, ell, var):
    """Host-side scalar: (T/N_GRID) * sum of log f over the grid points
    j in [-N/2, -P) that the on-device NS2-derived grid (j in [-P, 2P))
    does not cover.  Pure hyperparameter function, exact in f64."""
    j = np.arange(-N_GRID // 2, -P)
    th = 2.0 * np.pi * j / N_GRID
    s = np.zeros(j.shape)
    for m in (-2, -1, 0, 1, 2):
        s += np.exp(-((ell * (th - 2.0 * np.pi * m)) ** 2) / 2.0)
    f = sig2 + var * ell * math.sqrt(2.0 * math.pi) * s
    return float(T / N_GRID * np.sum(np.log(f)))



def _bias_arg(nc, cpool, mybir, value, tag):
    """Activation bias: floats 0/1 pass through (registered const APs);
    anything else gets a dedicated [P,1] memset tile."""
    if float(value) in (0.0, 1.0):
        return float(value)
    t = cpool.tile([P, 1], mybir.dt.float32, tag=tag)
    nc.vector.memset(t[:], float(value))
    return t[:]

def _emit_one_u(
    nc, tc, cpool, wpool, ppool, mybir,
    y_dram, gtab_dram, out_dram, dbg,
    sig2, ell, var, ci, variant="U13",
):
    """Variant U: fully unrolled recurrence (no For_i), gamma as immediate
    floats, static buffer rotation (y loaded once; the second step
    subtracts q_0 directly from the preserved y tile), Szego grid recycled
    from the squared NS2 iota, single merged reduce."""
    f32 = mybir.dt.float32
    AF = mybir.ActivationFunctionType
    OP = mybir.AluOpType

    n_deg = int(variant[1:])
    gam, lam_lo, lam_hi = _ls_poly_U(sig2, ell, var, n_deg)
    sc2 = 4.0 / (lam_hi - lam_lo)
    sh2 = -2.0 * (lam_hi + lam_lo) / (lam_hi - lam_lo)
    corr = _szego_corr(sig2, ell, var) + _szego_missing(sig2, ell, var)
    # gamma0-normalized schedule: the device accumulates x~ = x/gamma0
    # (so step 1 can fuse the x init: x~ = y + g1 q_1); gamma0 reappears
    # in the ub scalar and in the ones column of the partition reduce.
    g0 = float(gam[0])
    gn = [float(g / gam[0]) for g in gam]

    # big: vy | vb | vc | xs slots (66 cols each, window 1..65)
    big = wpool.tile([P, 4 * SW], f32, tag=f"big{ci}")
    nc.vector.memset(big[:], 0.0)
    vy = big[:, 0 * SW : 1 * SW]
    vb = big[:, 1 * SW : 2 * SW]
    vc = big[:, 2 * SW : 3 * SW]
    xt = big[:, 3 * SW : 4 * SW]
    yw = vy[:, 1 : 1 + NBLK]
    nc.sync.dma_start(yw, y_dram.rearrange("(b r) -> r b", b=NBLK))
    xw = xt[:, 1 : 1 + NBLK]

    # NS2 build; the squared iota doubles as the Szego grid source
    NS2 = cpool.tile([P, 3, P], f32, tag=f"NS2{ci}")
    nc.gpsimd.iota(
        NS2[:], pattern=[[P, 3], [-1, P]], base=-P, channel_multiplier=1,
        allow_small_or_imprecise_dtypes=True,
    )
    nc.scalar.activation(NS2[:], NS2[:], AF.Square)
    # Szego: grid j = p + 128(m-1) at NS2[:, :, 0] (values j^2 after
    # Square); log f = ln(v l sqrt(2pi) exp(-(2pi j/N)^2 l^2/2) + sig2),
    # prescaled by T/N and parked in t0 cols 64..66 for the merged reduce.
    t0 = wpool.tile([P, NBLK + 3], f32, tag=f"t0{ci}")
    gl = t0[:, NBLK : NBLK + 3]
    th_sc = (2.0 * math.pi / N_GRID) ** 2 * ell * ell / 2.0
    nc.scalar.activation(gl, NS2[:, :, 0], AF.Exp, scale=float(-th_sc))
    nc.scalar.activation(
        gl, gl, AF.Ln,
        scale=float(var * ell * math.sqrt(2.0 * math.pi)),
        bias=_bias_arg(nc, cpool, mybir, sig2, f"sgb{ci}"),
    )
    nc.vector.tensor_scalar(
        gl, gl, float(-0.5 * T / N_GRID / g0), None, op0=OP.mult
    )
    # rest of NS2 build
    nc.scalar.activation(
        NS2[:], NS2[:], AF.Exp, scale=float(-1.0 / (2.0 * ell * ell))
    )
    nc.vector.tensor_scalar(
        NS2[:], NS2[:], float(sc2 * var), None, op0=OP.mult
    )
    nc.gpsimd.affine_select(
        out=NS2[:, 1, :], in_=NS2[:, 1, :],
        compare_op=mybir.AluOpType.not_equal,
        fill=float(sc2 * (var + sig2) + sh2),
        base=0, pattern=[[-1, P]], channel_multiplier=1,
    )

    W_ps = ppool.tile([P, NBLK], f32, tag="W_ps")

    def matvec(src):
        for m in range(3):
            nc.tensor.matmul(
                W_ps[:], NS2[:, m, :], src[:, m : m + NBLK],
                start=(m == 0), stop=(m == 2), skip_group_check=True,
            )

    # unrolled recurrence; q_0 lives only in vy (read-only):
    #   step 1: q_1 = (2As) vy - vc(0) -> vc
    #   step 2: q_2 = (2As) vc - vy    -> vb   (in1 = vy, out = vb)
    #   step m>2: standard (vc, vb) rotation
    for m in range(1, n_deg):
        if m == 1:
            srcv, subv, dstv = vy, vc, vc
        elif m == 2:
            srcv, subv, dstv = vc, vy, vb
        elif m % 2 == 1:
            srcv, subv, dstv = vb, vc, vc
        else:
            srcv, subv, dstv = vc, vb, vb
        matvec(srcv)
        nc.vector.scalar_tensor_tensor(
            dstv[:, 1 : 1 + NBLK], in0=W_ps[:], scalar=1.0,
            in1=subv[:, 1 : 1 + NBLK], op0=OP.mult, op1=OP.subtract,
        )
        # m == 1 fuses the x init: x~ = 1.0*y + gn[1] q_1
        nc.vector.scalar_tensor_tensor(
            xw, in0=dstv[:, 1 : 1 + NBLK], scalar=float(gn[m]),
            in1=(yw if m == 1 else xw), op0=OP.mult, op1=OP.add,
        )

    # quad: t0[:, 0:64] = -0.5 * x * (2y - A x)
    #   ua = sh2*x - (2As)x;  ub = (-0.5/sc2)*ua - y;  tq = ub * x
    matvec(xt)
    tq = t0[:, 0:NBLK]
    nc.vector.scalar_tensor_tensor(
        tq, in0=xw, scalar=float(sh2), in1=W_ps[:],
        op0=OP.mult, op1=OP.subtract,
    )
    nc.vector.scalar_tensor_tensor(
        tq, in0=tq, scalar=float(-0.5 * g0 / sc2), in1=yw,
        op0=OP.mult, op1=OP.subtract,
    )
    nc.vector.tensor_tensor(tq, tq, xw, op=OP.mult)

    # merged reduce; the ones column carries gamma0 so that
    # g0 * (tq~ | gl~) sums to -0.5*quad - (T/2N)*sum(log f)
    red = wpool.tile([P, 1], f32, tag=f"red{ci}")
    nc.vector.tensor_reduce(red[:], t0[:], axis=mybir.AxisListType.X, op=OP.add)
    ones = cpool.tile([P, 1], f32, tag=f"ones{ci}")
    nc.vector.memset(ones[:], float(g0))
    out_ps = ppool.tile([1, 1], f32, tag="out_ps")
    nc.tensor.matmul(
        out_ps[:], ones[:], red[:], start=True, stop=True,
        skip_group_check=True,
    )
    fin = wpool.tile([1, 1], f32, tag=f"fin{ci}")
    nc.vector.tensor_scalar(
        fin[:], out_ps[:], 1.0, float(-0.5 * corr), op0=OP.mult, op1=OP.add
    )
    nc.sync.dma_start(out_dram[:, ci : ci + 1], fin[:])
